# revision 30
# baseline (speedup 1.0000x reference)
"""Trainium2 Bass kernel for a dense transformer block (pre-LN, 16-head causal
attention + 3x FFN), distributed over 8 NeuronCores.

Sharding: tensor-parallel over heads (2 heads/core, both batch elements on
every core) for LN1/QKV/attention; one 8-core AllToAll redistributes the
per-head attention context to token-parallel shards (512 tokens/core) for the
output projection, LN2 and the FFN.  Matmuls run in bf16 with f32 PSUM
accumulation; the residual stream stays f32.

v2 changes vs baseline:
- LN1 stats computed replicated on every core (no AllGather): sum(x) on the
  vector engine, sum(x^2) via scalar Square+accum_out, so no collective sits
  in front of the QKV matmuls.
- A dummy 32-byte AllGather is issued first so the one-time comm-init /
  launch-skew barrier (~63us) overlaps stage A+B compute instead of stalling.
- Softmax normalization uses reciprocal_approx_fast (vector `reciprocal` was
  3.1us per call) and gpsimd partition_broadcast instead of PE broadcast
  matmuls.
- Causal N-restriction: score/exp/AV tiles in the diagonal 512-block only
  cover valid query columns (saves ~15% of attention PE+scalar work).
- Stage C: bo/b2 bias matmuls folded into fused scalar_tensor_tensor ops;
  LN2 sums taken directly from f32/bf16 operands; mean/inv broadcast on
  gpsimd; Wo weights + xTs preloaded during attention.
"""

import numpy as np
import ml_dtypes

B, T, C = 2, 2048, 1024
NH, H = 16, 64
FF = 3 * C
EPS = 1e-6
N_CORES = 8
TT = B * T            # 4096 tokens processed per core (head-parallel phase)
TS = TT // N_CORES    # 512 tokens per core (token-parallel phase)
HPC = NH // N_CORES   # 2 heads per core
HD2 = HPC * H         # 128

BF16 = ml_dtypes.bfloat16

_BUILT = {}

NT = TT // 128        # 32 token tiles
NKC = C // 128        # 8 channel k-tiles
NMF = FF // 128       # 24 ff tiles


def _build():
    import concourse.bacc as bacc
    import concourse.mybir as mybir
    import concourse.tile as tile
    dt = mybir.dt
    alu = mybir.AluOpType
    act = mybir.ActivationFunctionType

    nc = bacc.Bacc("TRN2", target_bir_lowering=False, debug=False,
                   num_devices=N_CORES)

    # ----- kernel I/O (per-core shards) -----
    p_xT = nc.declare_dram_parameter("p_xT", [C, TT], dt.bfloat16, isOutput=False)
    p_xTs = nc.declare_dram_parameter("p_xTs", [C, TS], dt.float32, isOutput=False)
    p_wq = nc.declare_dram_parameter("p_wq", [C, HD2], dt.bfloat16, isOutput=False)
    p_wk = nc.declare_dram_parameter("p_wk", [C, HD2], dt.bfloat16, isOutput=False)
    p_wv = nc.declare_dram_parameter("p_wv", [C, HD2], dt.bfloat16, isOutput=False)
    p_cq = nc.declare_dram_parameter("p_cq", [1, HD2], dt.bfloat16, isOutput=False)
    p_ck = nc.declare_dram_parameter("p_ck", [1, HD2], dt.bfloat16, isOutput=False)
    p_cv = nc.declare_dram_parameter("p_cv", [1, HD2], dt.bfloat16, isOutput=False)
    p_woblk = nc.declare_dram_parameter("p_woblk", [NKC, C, 128], dt.bfloat16, isOutput=False)
    p_boc = nc.declare_dram_parameter("p_boc", [128, NKC], dt.float32, isOutput=False)
    p_w1blk = nc.declare_dram_parameter("p_w1blk", [NMF, C, 128], dt.bfloat16, isOutput=False)
    p_b1c = nc.declare_dram_parameter("p_b1c", [128, NMF], dt.float32, isOutput=False)
    p_w2blk = nc.declare_dram_parameter("p_w2blk", [NKC, FF, 128], dt.bfloat16, isOutput=False)
    p_b2c = nc.declare_dram_parameter("p_b2c", [128, NKC], dt.float32, isOutput=False)
    p_maskd = nc.declare_dram_parameter("p_maskd", [128, 128], dt.bfloat16, isOutput=False)
    p_ident = nc.declare_dram_parameter("p_ident", [128, 128], dt.bfloat16, isOutput=False)
    p_out = nc.declare_dram_parameter("p_out", [C, TS], dt.float32, isOutput=True)

    with tile.TileContext(nc, num_cores=N_CORES) as tc:
        with (
            tc.tile_pool(name="persist", bufs=1) as pp,
            tc.tile_pool(name="dram", bufs=1, space="DRAM") as pdram,
        ):
            # ---- dummy first collective: absorbs the one-time comm barrier
            dum_s = pp.tile([1, 16], dt.bfloat16)
            nc.vector.memset(dum_s[:], 0.0)
            dum_in = pdram.tile([1, 16], dt.bfloat16)
            dum_out = pdram.tile([N_CORES, 16], dt.bfloat16)
            nc.sync.dma_start(dum_in[:], dum_s[:])
            nc.gpsimd.collective_compute(
                "AllGather", alu.bypass,
                replica_groups=[list(range(N_CORES))],
                ins=[dum_in.opt()],
                outs=[dum_out.opt()],
            )

            # ------------- persistent constants & activation tensors -------------
            ident = pp.tile([128, 128], dt.bfloat16)
            nc.scalar.dma_start(ident[:], p_ident[:])
            maskd = pp.tile([128, 128], dt.bfloat16)
            nc.scalar.dma_start(maskd[:], p_maskd[:])

            cq = pp.tile([1, HD2], dt.bfloat16)
            nc.scalar.dma_start(cq[:], p_cq[:])
            ck = pp.tile([1, HD2], dt.bfloat16)
            nc.scalar.dma_start(ck[:], p_ck[:])
            cv = pp.tile([1, HD2], dt.bfloat16)
            nc.scalar.dma_start(cv[:], p_cv[:])
            ones_row128 = pp.tile([1, 128], dt.bfloat16)
            nc.vector.memset(ones_row128[:], 1.0)

            with tc.tile_pool(name="ab", bufs=1) as pab:
                qT = pab.tile([128, TT], dt.bfloat16)
                kT = pab.tile([128, TT], dt.bfloat16)
                v = pab.tile([128, NT, 2, 65], dt.bfloat16)
                ctxT = pab.tile([128, TT], dt.bfloat16)

                # ---------------- stage A: LN1 stats (replicated) + QKV --------
                with (
                    tc.tile_pool(name="xtpool", bufs=1) as pxt,
                    tc.tile_pool(name="sqscr", bufs=2) as psc,
                    tc.tile_pool(name="strow", bufs=2) as pstr,
                    tc.tile_pool(name="apsum", bufs=4, space="PSUM") as pps_a,
                    tc.tile_pool(name="apsum1", bufs=1, space="PSUM") as pps_a1,
                    tc.tile_pool(name="stpsum", bufs=1, space="PSUM") as pps_st,
                ):
                    # QKV weights: [C, 128] -> [128, 8, 128] (k-tile at [:, k, :])
                    wq = pxt.tile([128, NKC, HD2], dt.bfloat16)
                    nc.scalar.dma_start(wq[:], p_wq.ap().rearrange("(k p) h -> p k h", p=128))
                    wk = pxt.tile([128, NKC, HD2], dt.bfloat16)
                    nc.scalar.dma_start(wk[:], p_wk.ap().rearrange("(k p) h -> p k h", p=128))
                    wv = pxt.tile([128, NKC, HD2], dt.bfloat16)
                    nc.scalar.dma_start(wv[:], p_wv.ap().rearrange("(k p) h -> p k h", p=128))

                    pj0 = pps_a1.tile([128, 512], dt.float32, tag="invb")
                    for _ in range(30):
                        nc.tensor.matmul(pj0[:, 0:128], ident[:], ident[:],
                                         start=True, stop=True)

                    inv_b = pxt.tile([128, TT], dt.bfloat16)
                    vT = pxt.tile([128, TT], dt.bfloat16)
                    # rows_neg [1, TT]: -mu per token (be1 == 0 so the std+eps
                    # correction row vanishes); inv_row [1, TT]: 1/(std+eps)
                    rows_neg = pxt.tile([1, TT], dt.bfloat16)
                    inv_row = pxt.tile([1, TT], dt.bfloat16)

                    # x^T resident for the QKV matmuls, DMA'd per token-chunk
                    xT = pxt.tile([128, NKC, TT], dt.bfloat16)
                    for ch in range(TT // 512):
                        nc.sync.dma_start(
                            xT[:, :, 512 * ch:512 * (ch + 1)],
                            p_xT.ap()[:, 512 * ch:512 * (ch + 1)].rearrange(
                                "(k p) t -> p k t", p=128))

                    # LN1 stats replicated on every core, computed from xT in
                    # row form: mean/meansq via ones-column PE matmul sums
                    # (no token-major x copy, no stat transposes).
                    iscb = pxt.tile([128, 1], dt.bfloat16)
                    nc.vector.memset(iscb[:], 1.0 / C)
                    nc.vector.memset(v[:, :, :, 64], 1.0)

                    for ch in range(TT // 512):
                        sl = slice(512 * ch, 512 * (ch + 1))
                        ps_mu = pps_st.tile([1, 512], dt.float32, tag="mu")
                        for k in range(NKC):
                            nc.tensor.matmul(ps_mu[:], iscb[:], xT[:, k, sl],
                                             start=(k == 0), stop=(k == NKC - 1))
                        scr2 = psc.tile([128, NKC, 512], dt.bfloat16, tag="scr")
                        nc.scalar.square(scr2[:], xT[:, :, sl])
                        ps_sq = pps_st.tile([1, 512], dt.float32, tag="sq")
                        for k in range(NKC):
                            nc.tensor.matmul(ps_sq[:], iscb[:], scr2[:, k, :],
                                             start=(k == 0), stop=(k == NKC - 1))
                        mu_row = pstr.tile([1, 512], dt.float32, tag="mur")
                        nc.vector.tensor_copy(mu_row[:], ps_mu[:])
                        nc.vector.tensor_scalar(rows_neg[:, sl], ps_mu[:], -1.0,
                                                None, alu.mult)
                        m2r = pstr.tile([1, 512], dt.float32, tag="m2r")
                        nc.vector.tensor_tensor(m2r[:], mu_row[:], mu_row[:],
                                                alu.mult)
                        varr = pstr.tile([1, 512], dt.float32, tag="varr")
                        nc.vector.tensor_tensor(varr[:], ps_sq[:], m2r[:],
                                                alu.subtract)
                        stdr = pstr.tile([1, 512], dt.float32, tag="stdr")
                        nc.scalar.activation(stdr[:], varr[:], act.Sqrt,
                                             scale=float(C) / (C - 1))
                        nc.vector.tensor_scalar(stdr[:], stdr[:], EPS, None,
                                                alu.add)
                        invr = pstr.tile([1, 512], dt.float32, tag="invr")
                        nc.vector.reciprocal_approx_fast(invr[:], stdr[:])
                        nc.vector.tensor_copy(inv_row[:, sl], invr[:])
                        # inv broadcast down partitions: 1 PE matmul + copy
                        pbi = pps_a1.tile([128, 512], dt.float32, tag="invb")
                        nc.tensor.matmul(pbi[:], ones_row128[:],
                                         inv_row[0:1, sl], start=True, stop=True)
                        nc.vector.tensor_copy(inv_b[:, sl], pbi[:])

                        # QKV for this chunk
                        for (nm, w, cw, dst) in (("q", wq, cq, qT), ("k", wk, ck, kT),
                                                 ("v", wv, cv, vT)):
                            ps = pps_a.tile([128, 512], dt.float32,
                                            name=f"ps{nm}", tag="qkv")
                            for k in range(NKC):
                                nc.tensor.matmul(ps[:], w[:, k, :], xT[:, k, sl],
                                                 start=(k == 0), stop=False)
                            nc.tensor.matmul(ps[:], cw[:], rows_neg[:, sl],
                                             start=False, stop=True)
                            nc.vector.tensor_tensor(dst[:, sl], ps[:], inv_b[:, sl],
                                                    alu.mult)

                        # v_aug [s, tile, head, 65] via PE transposes (paired)
                        for pr in range(2):
                            i0 = 4 * ch + 2 * pr
                            pvt = pps_a1.tile([128, 2, 128], dt.bfloat16, tag="vtp")
                            for u in range(2):
                                nc.tensor.transpose(
                                    pvt[:, u, :],
                                    vT[:, 128 * (i0 + u):128 * (i0 + u + 1)],
                                    ident[:])
                            nc.scalar.copy(
                                v[:, i0:i0 + 2, :, 0:64],
                                pvt[:].rearrange("p u (h d) -> p u h d", h=2))

                # ------- preload stage-C data (fills during attention) -------
                with (
                    tc.tile_pool(name="wc", bufs=1) as pwc,
                ):
                    wo_sb = pwc.tile([128, NKC, NKC, 128], dt.bfloat16)
                    for mc in range(NKC):
                        nc.scalar.dma_start(
                            wo_sb[:, mc],
                            p_woblk[mc].rearrange("(k p) c -> p k c", p=128))
                    xTs = pwc.tile([128, NKC, TS], dt.float32)
                    nc.scalar.dma_start(xTs[:], p_xTs.ap().rearrange("(k p) t -> p k t", p=128))
                    boc = pwc.tile([128, NKC], dt.float32)
                    nc.sync.dma_start(boc[:], p_boc[:])
                    b2c = pwc.tile([128, NKC], dt.float32)
                    nc.sync.dma_start(b2c[:], p_b2c[:])
                    b1c = pwc.tile([128, NMF], dt.float32)
                    nc.sync.dma_start(b1c[:], p_b1c[:])
                    isc32 = pwc.tile([128, 1], dt.float32)
                    nc.vector.memset(isc32[:], 1.0 / C)
                    iscb = pwc.tile([128, 1], dt.bfloat16)
                    nc.vector.memset(iscb[:], 1.0 / C)

                    cc_in = pdram.tile([N_CORES, 128, TS], dt.bfloat16)
                    cc_out = pdram.tile([N_CORES, 128, TS], dt.bfloat16)

                    # ---------------- stage B: attention ----------------
                    with (
                        tc.tile_pool(name="exps", bufs=6) as pexp,
                        tc.tile_pool(name="attsb", bufs=6) as pat,
                        tc.tile_pool(name="scpsum", bufs=2, space="PSUM") as pps_sc,
                        tc.tile_pool(name="ctxpsum", bufs=2, space="PSUM") as pps_ctx,
                    ):
                        for b in range(B):
                            for qt in range(T // 512):
                                G = b * T + 512 * qt
                                gsl = slice(G, G + 512)
                                nj = 4 * qt + 4
                                pc = [pps_ctx.tile([65, 512], dt.float32,
                                                   name=f"pc{h}", tag=f"ctx{h}")
                                      for h in range(2)]
                                ets = []
                                for j in range(nj):
                                    st = b * (T // 128) + j   # global s-tile index
                                    off = max(0, j - (nj - 4))
                                    o = 128 * off
                                    # both heads' scores into one 2-bank psum
                                    # tile; the two K=64 matmuls use disjoint
                                    # PE row groups and run concurrently.
                                    ps = pps_sc.tile([128, 2, 512], dt.float32,
                                                     tag="sc")
                                    for h in range(2):
                                        hsl = slice(64 * h, 64 * (h + 1))
                                        nc.tensor.matmul(
                                            ps[:, h, o:512],
                                            kT[hsl, 128 * st:128 * (st + 1)],
                                            qT[hsl, G + o:G + 512],
                                            start=True, stop=True)
                                    # one exp over both heads (amortizes the
                                    # ~300ns ACT fixed cost per instruction)
                                    et = pexp.tile([128, 2, 512], dt.bfloat16,
                                                   tag="et")
                                    nc.scalar.activation(
                                        et[:, :, o:512], ps[:, :, o:512],
                                        act.Exp, scale=1.0 / float(np.sqrt(H)))
                                    if j >= nj - 4:
                                        for h in range(2):
                                            nc.vector.tensor_tensor(
                                                et[:, h, o:o + 128],
                                                et[:, h, o:o + 128],
                                                maskd[:], alu.mult)
                                    ets.append((et, o))
                                    # software pipeline: AV for tile j-1 after scores of j
                                    if j > 0:
                                        pe2, po = ets[j - 1]
                                        for h in range(2):
                                            nc.tensor.matmul(
                                                pc[h][:, po:512],
                                                v[:, b * (T // 128) + j - 1, h, :],
                                                pe2[:, h, po:512],
                                                start=(j - 1 == 0), stop=False)
                                pe2, po = ets[nj - 1]
                                for h in range(2):
                                    nc.tensor.matmul(
                                        pc[h][:, po:512],
                                        v[:, b * (T // 128) + nj - 1, h, :],
                                        pe2[:, h, po:512],
                                        start=(nj == 1), stop=True)
                                # normalize by Z (row 64 of each ctx psum).
                                # zbf row = [1/Z_h0 | 1/Z_h1]; broadcast must
                                # write from partition 0, so head h reads
                                # zbf[64h:64h+64, 512h:512h+512].
                                zr = pat.tile([1, 2, 512], dt.float32, tag="zr")
                                nc.vector.tensor_copy(zr[:, 0, :], pc[0][64:65, :])
                                nc.vector.tensor_copy(zr[:, 1, :], pc[1][64:65, :])
                                zi = pat.tile([1, 2, 512], dt.float32, tag="zi")
                                nc.vector.reciprocal_approx_fast(zi[:], zr[:])
                                zib = pat.tile([1, 1024], dt.bfloat16, tag="zib")
                                nc.vector.tensor_copy(
                                    zib[:].rearrange("p (h t) -> p h t", h=2), zi[:])
                                zbf = pat.tile([128, 1024], dt.bfloat16, tag="zb")
                                nc.gpsimd.partition_broadcast(zbf[:], zib[0:1, :])
                                for h in range(2):
                                    nc.vector.tensor_tensor(
                                        ctxT[64 * h:64 * (h + 1), gsl],
                                        pc[h][0:64, :],
                                        zbf[64 * h:64 * (h + 1),
                                            512 * h:512 * (h + 1)],
                                        alu.mult)
                                # ship this token-chunk to its owner core
                                nc.sync.dma_start(cc_in[b * 4 + qt], ctxT[:, gsl])

                    # ---------------- AllToAll: heads -> tokens ----------------
                    nc.gpsimd.collective_compute(
                        "AllToAll", alu.bypass,
                        replica_groups=[list(range(N_CORES))],
                        ins=[cc_in.opt()],
                        outs=[cc_out.opt()],
                    )

                    # ---------------- stage C: Wo + LN2 + FFN ----------------
                    with (
                        tc.tile_pool(name="postsb", bufs=1) as pq,
                        tc.tile_pool(name="wstream", bufs=3) as pw,
                        tc.tile_pool(name="evict", bufs=3) as pev,
                        tc.tile_pool(name="ln2tmp", bufs=1) as pl2,
                        tc.tile_pool(name="ffpsum", bufs=4, space="PSUM") as pps_ff,
                        tc.tile_pool(name="cpsum", bufs=1, space="PSUM") as pps_c,
                    ):
                        pj = pps_ff.tile([128, 128], dt.float32, tag="ff")
                        for _ in range(144):
                            nc.tensor.matmul(pj[:], ident[:], ident[:],
                                             start=True, stop=True)
                        sq0 = pq.tile([1, 16], dt.float32)
                        nc.vector.memset(sq0[:], 1.0)
                        sq1 = pq.tile([1, 16], dt.float32)
                        nc.scalar.activation(sq1[:], sq0[:], act.Sqrt)

                        ctxF = pq.tile([128, NKC, TS], dt.bfloat16)
                        for j in range(N_CORES):
                            eng = nc.sync if j % 2 == 0 else nc.scalar
                            eng.dma_start(ctxF[:, j, :], cc_out[j])

                        r2T = pq.tile([128, NKC, TS], dt.float32)
                        sq = pl2.tile([128, NKC, TS], dt.bfloat16)
                        ps1 = pps_c.tile([1, TS], dt.float32, tag="s1")
                        ps2 = pps_c.tile([1, TS], dt.float32, tag="s2")
                        for mc in range(NKC):
                            ps = pps_ff.tile([128, TS], dt.float32, tag="ff")
                            for k in range(NKC):
                                nc.tensor.matmul(ps[:], wo_sb[:, mc, k, :],
                                                 ctxF[:, k, :],
                                                 start=(k == 0), stop=(k == NKC - 1))
                            nc.vector.scalar_tensor_tensor(
                                r2T[:, mc, :], ps[:], boc[:, mc:mc + 1],
                                xTs[:, mc, :], alu.add, alu.add)
                            nc.scalar.square(sq[:, mc, :], r2T[:, mc, :])
                            nc.tensor.matmul(ps1[:], isc32[:], r2T[:, mc, :],
                                             start=(mc == 0), stop=(mc == NKC - 1))
                            nc.tensor.matmul(ps2[:], iscb[:], sq[:, mc, :],
                                             start=(mc == 0), stop=(mc == NKC - 1))

                        # ---- LN2 over the channel (partition) dim ----
                        mu2f = pl2.tile([1, TS], dt.float32)
                        nc.vector.tensor_copy(mu2f[:], ps1[:])
                        varf2 = pl2.tile([1, TS], dt.float32)
                        nc.vector.tensor_tensor(varf2[:], mu2f[:], mu2f[:], alu.mult)
                        nc.vector.tensor_tensor(varf2[:], ps2[:], varf2[:],
                                                alu.subtract)
                        stdf2 = pl2.tile([1, TS], dt.float32)
                        nc.scalar.activation(stdf2[:], varf2[:], act.Sqrt,
                                             scale=float(C) / (C - 1))
                        nc.vector.tensor_scalar(stdf2[:], stdf2[:], EPS, None,
                                                alu.add)
                        inv2 = pl2.tile([1, TS], dt.float32)
                        nc.vector.reciprocal_approx_fast(inv2[:], stdf2[:])
                        mu2row = pl2.tile([1, TS], dt.bfloat16)
                        nc.vector.tensor_copy(mu2row[:], mu2f[:])
                        inv2row = pl2.tile([1, TS], dt.bfloat16)
                        nc.vector.tensor_copy(inv2row[:], inv2[:])
                        pmb = pps_c.tile([128, TS], dt.float32, tag="bc1")
                        nc.tensor.matmul(pmb[:], ones_row128[:], mu2row[0:1, :],
                                         start=True, stop=True)
                        m2b = pl2.tile([128, TS], dt.bfloat16)
                        nc.vector.tensor_copy(m2b[:], pmb[:])
                        pib = pps_c.tile([128, TS], dt.float32, tag="bc2")
                        nc.tensor.matmul(pib[:], ones_row128[:], inv2row[0:1, :],
                                         start=True, stop=True)
                        i2b = pl2.tile([128, TS], dt.bfloat16)
                        nc.vector.tensor_copy(i2b[:], pib[:])

                        xn2T = pq.tile([128, NKC, TS], dt.bfloat16)
                        for mc in range(NKC):
                            tmp = pev.tile([128, TS], dt.bfloat16, tag="xtmp")
                            eng = nc.gpsimd if mc % 2 else nc.vector
                            eng.tensor_tensor(tmp[:], r2T[:, mc, :], m2b[:],
                                              alu.subtract)
                            nc.vector.tensor_tensor(xn2T[:, mc, :], tmp[:], i2b[:],
                                                    alu.mult)

                        # ---- FFN ----
                        hT = pq.tile([128, NMF, TS], dt.bfloat16)
                        for mf in range(NMF):
                            w1_blk = pw.tile([128, NKC, 128], dt.bfloat16, tag="w1")
                            nc.sync.dma_start(
                                w1_blk[:],
                                p_w1blk[mf].rearrange("(k p) f -> p k f", p=128))
                            ps = pps_ff.tile([128, TS], dt.float32, tag="ff")
                            for k in range(NKC):
                                nc.tensor.matmul(ps[:], w1_blk[:, k, :],
                                                 xn2T[:, k, :],
                                                 start=(k == 0), stop=(k == NKC - 1))
                            nc.scalar.activation(hT[:, mf, :], ps[:],
                                                 act.Relu,
                                                 bias=b1c[:, mf:mf + 1])

                        for mc in range(NKC):
                            w2_blk = pw.tile([128, NMF, 128], dt.bfloat16, tag="w2")
                            nc.sync.dma_start(
                                w2_blk[:],
                                p_w2blk[mc].rearrange("(k p) c -> p k c", p=128))
                            ps = pps_ff.tile([128, TS], dt.float32, tag="ff")
                            for k in range(NMF):
                                nc.tensor.matmul(ps[:], w2_blk[:, k, :], hT[:, k, :],
                                                 start=(k == 0), stop=(k == NMF - 1))
                            ot = pev.tile([128, TS], dt.float32, tag="ot")
                            nc.vector.scalar_tensor_tensor(
                                ot[:], ps[:], b2c[:, mc:mc + 1], r2T[:, mc, :],
                                alu.add, alu.add)
                            nc.sync.dma_start(p_out[128 * mc:128 * (mc + 1), :], ot[:])

    nc.compile()
    return nc


def _host_prep(inputs):
    """Fold layernorm affine params into weights; build per-core input maps."""
    x = np.asarray(inputs["x"], np.float32)
    Wq = np.asarray(inputs["Wq"], np.float32)
    Wk = np.asarray(inputs["Wk"], np.float32)
    Wv = np.asarray(inputs["Wv"], np.float32)
    Wo = np.asarray(inputs["Wo"], np.float32)
    bo = np.asarray(inputs["bo"], np.float32)
    W1 = np.asarray(inputs["W1"], np.float32)
    b1 = np.asarray(inputs["b1"], np.float32)
    W2 = np.asarray(inputs["W2"], np.float32)
    b2 = np.asarray(inputs["b2"], np.float32)
    g1 = np.asarray(inputs["g1"], np.float32)
    be1 = np.asarray(inputs["be1"], np.float32)
    g2 = np.asarray(inputs["g2"], np.float32)
    be2 = np.asarray(inputs["be2"], np.float32)

    xf = x.reshape(TT, C)                      # both batches stacked
    xT = np.ascontiguousarray(xf.T)            # [C, TT]

    # the kernel folds LN1 as (x@Weff - mu*colsum(Weff)) * inv; the be1 bias
    # term would need a second correction row — this problem has be1 == 0.
    assert np.abs(be1).max() == 0.0

    def fold_qkv(W):
        Weff = g1[:, None] * W                  # [NH, C, H] with g1 on C
        Weff = np.ascontiguousarray(np.transpose(Weff, (1, 0, 2)))  # [C, NH, H]
        colsum = Weff.sum(axis=0)               # [NH, H]
        return Weff, colsum

    Wq_e, csq = fold_qkv(Wq)
    Wk_e, csk = fold_qkv(Wk)
    Wv_e, csv = fold_qkv(Wv)

    woT = np.ascontiguousarray(Wo.T)            # [NH*H, C]
    w1T = np.ascontiguousarray(g2[:, None] * W1.T)   # [C, FF]
    b1_eff = b1 + be2 @ W1.T                         # [FF]
    w2T = np.ascontiguousarray(W2.T)            # [FF, C]

    # blocked weights: [nblocks, K, 128] with contiguous [K, 128] blocks
    woblk = np.ascontiguousarray(
        woT.reshape(C, NKC, 128).transpose(1, 0, 2))
    w1blk = np.ascontiguousarray(
        w1T.reshape(C, NMF, 128).transpose(1, 0, 2))
    w2blk = np.ascontiguousarray(
        w2T.reshape(FF, NKC, 128).transpose(1, 0, 2))

    tq = np.arange(128)[None, :]
    s = np.arange(128)[:, None]
    maskd = (s <= tq).astype(BF16)

    shared = {
        "p_xT": xT.astype(BF16),
        "p_woblk": woblk.astype(BF16),
        "p_boc": np.ascontiguousarray(
            bo.reshape(NKC, 128).T).astype(np.float32),
        "p_w1blk": w1blk.astype(BF16),
        "p_b1c": np.ascontiguousarray(
            b1_eff.reshape(NMF, 128).T).astype(np.float32),
        "p_w2blk": w2blk.astype(BF16),
        "p_b2c": np.ascontiguousarray(
            b2.reshape(NKC, 128).T).astype(np.float32),
        "p_maskd": maskd,
        "p_ident": np.eye(128, dtype=np.float32).astype(BF16),
    }

    in_maps = []
    for r in range(N_CORES):
        h0 = HPC * r
        hs = slice(h0, h0 + HPC)
        b_r, s_r = divmod(r, N_CORES // B)
        tok = slice(s_r * TS, (s_r + 1) * TS)
        xTs = np.ascontiguousarray(x[b_r].T[:, tok])
        m = dict(shared)
        m["p_xTs"] = xTs.astype(np.float32)
        m["p_wq"] = np.ascontiguousarray(
            Wq_e[:, hs, :].reshape(C, HD2)).astype(BF16)
        m["p_wk"] = np.ascontiguousarray(
            Wk_e[:, hs, :].reshape(C, HD2)).astype(BF16)
        m["p_wv"] = np.ascontiguousarray(
            Wv_e[:, hs, :].reshape(C, HD2)).astype(BF16)
        m["p_cq"] = csq[hs].reshape(1, HD2).astype(BF16)
        m["p_ck"] = csk[hs].reshape(1, HD2).astype(BF16)
        m["p_cv"] = csv[hs].reshape(1, HD2).astype(BF16)
        in_maps.append(m)
    return in_maps


def kernel(**inputs) -> np.ndarray:
    from concourse.bass_utils import run_bass_kernel_spmd

    if "nc" not in _BUILT:
        _BUILT["nc"] = _build()
    nc = _BUILT["nc"]

    in_maps = _host_prep(inputs)
    res = run_bass_kernel_spmd(nc, in_maps, core_ids=list(range(N_CORES)))

    out = np.empty((B, T, C), np.float32)
    for r in range(N_CORES):
        b_r, s_r = divmod(r, N_CORES // B)
        out[b_r, s_r * TS:(s_r + 1) * TS, :] = res.results[r]["p_out"].T
    return out


# revision 32
# speedup vs baseline: 1.0415x; 1.0415x over previous
"""Trainium2 Bass kernel for a dense transformer block (pre-LN, 16-head causal
attention + 3x FFN), distributed over 8 NeuronCores.

Sharding: tensor-parallel over heads (2 heads/core, both batch elements on
every core) for LN1/QKV/attention; one 8-core AllToAll redistributes the
per-head attention context to token-parallel shards (512 tokens/core) for the
output projection, LN2 and the FFN.  Matmuls run in bf16 with f32 PSUM
accumulation; the residual stream stays f32.

v2 changes vs baseline:
- LN1 stats computed replicated on every core (no AllGather): sum(x) on the
  vector engine, sum(x^2) via scalar Square+accum_out, so no collective sits
  in front of the QKV matmuls.
- A dummy 32-byte AllGather is issued first so the one-time comm-init /
  launch-skew barrier (~63us) overlaps stage A+B compute instead of stalling.
- Softmax normalization uses reciprocal_approx_fast (vector `reciprocal` was
  3.1us per call) and gpsimd partition_broadcast instead of PE broadcast
  matmuls.
- Causal N-restriction: score/exp/AV tiles in the diagonal 512-block only
  cover valid query columns (saves ~15% of attention PE+scalar work).
- Stage C: bo/b2 bias matmuls folded into fused scalar_tensor_tensor ops;
  LN2 sums taken directly from f32/bf16 operands; mean/inv broadcast on
  gpsimd; Wo weights + xTs preloaded during attention.
"""

import numpy as np
import ml_dtypes

B, T, C = 2, 2048, 1024
NH, H = 16, 64
FF = 3 * C
EPS = 1e-6
N_CORES = 8
TT = B * T            # 4096 tokens processed per core (head-parallel phase)
TS = TT // N_CORES    # 512 tokens per core (token-parallel phase)
HPC = NH // N_CORES   # 2 heads per core
HD2 = HPC * H         # 128

BF16 = ml_dtypes.bfloat16

_BUILT = {}

NT = TT // 128        # 32 token tiles
NKC = C // 128        # 8 channel k-tiles
NMF = FF // 128       # 24 ff tiles


def _build():
    import concourse.bacc as bacc
    import concourse.mybir as mybir
    import concourse.tile as tile
    dt = mybir.dt
    alu = mybir.AluOpType
    act = mybir.ActivationFunctionType

    nc = bacc.Bacc("TRN2", target_bir_lowering=False, debug=False,
                   num_devices=N_CORES)

    # ----- kernel I/O (per-core shards) -----
    p_xT = nc.declare_dram_parameter("p_xT", [C, TT], dt.bfloat16, isOutput=False)
    p_xTs = nc.declare_dram_parameter("p_xTs", [C, TS], dt.float32, isOutput=False)
    p_wq = nc.declare_dram_parameter("p_wq", [C, HD2], dt.bfloat16, isOutput=False)
    p_wk = nc.declare_dram_parameter("p_wk", [C, HD2], dt.bfloat16, isOutput=False)
    p_wv = nc.declare_dram_parameter("p_wv", [C, HD2], dt.bfloat16, isOutput=False)
    p_cq = nc.declare_dram_parameter("p_cq", [1, HD2], dt.bfloat16, isOutput=False)
    p_ck = nc.declare_dram_parameter("p_ck", [1, HD2], dt.bfloat16, isOutput=False)
    p_cv = nc.declare_dram_parameter("p_cv", [1, HD2], dt.bfloat16, isOutput=False)
    p_woblk = nc.declare_dram_parameter("p_woblk", [NKC, C, 128], dt.bfloat16, isOutput=False)
    p_boc = nc.declare_dram_parameter("p_boc", [128, NKC], dt.float32, isOutput=False)
    p_w1blk = nc.declare_dram_parameter("p_w1blk", [NMF, C, 128], dt.bfloat16, isOutput=False)
    p_b1c = nc.declare_dram_parameter("p_b1c", [128, NMF], dt.float32, isOutput=False)
    p_w2blk = nc.declare_dram_parameter("p_w2blk", [NKC, FF, 128], dt.bfloat16, isOutput=False)
    p_b2c = nc.declare_dram_parameter("p_b2c", [128, NKC], dt.float32, isOutput=False)
    p_maskd = nc.declare_dram_parameter("p_maskd", [128, 128], dt.bfloat16, isOutput=False)
    p_ident = nc.declare_dram_parameter("p_ident", [128, 128], dt.bfloat16, isOutput=False)
    p_out = nc.declare_dram_parameter("p_out", [C, TS], dt.float32, isOutput=True)

    with tile.TileContext(nc, num_cores=N_CORES) as tc:
        with (
            tc.tile_pool(name="persist", bufs=1) as pp,
            tc.tile_pool(name="dram", bufs=1, space="DRAM") as pdram,
        ):
            # ---- dummy first collective: absorbs the one-time comm barrier
            dum_s = pp.tile([1, 16], dt.bfloat16)
            nc.vector.memset(dum_s[:], 0.0)
            dum_in = pdram.tile([1, 16], dt.bfloat16)
            dum_out = pdram.tile([N_CORES, 16], dt.bfloat16)
            nc.sync.dma_start(dum_in[:], dum_s[:])
            nc.gpsimd.collective_compute(
                "AllGather", alu.bypass,
                replica_groups=[list(range(N_CORES))],
                ins=[dum_in.opt()],
                outs=[dum_out.opt()],
            )

            # ------------- persistent constants & activation tensors -------------
            ident = pp.tile([128, 128], dt.bfloat16)
            nc.scalar.dma_start(ident[:], p_ident[:])
            maskd = pp.tile([128, 128], dt.bfloat16)
            nc.scalar.dma_start(maskd[:], p_maskd[:])

            cq = pp.tile([1, HD2], dt.bfloat16)
            nc.scalar.dma_start(cq[:], p_cq[:])
            ck = pp.tile([1, HD2], dt.bfloat16)
            nc.scalar.dma_start(ck[:], p_ck[:])
            cv = pp.tile([1, HD2], dt.bfloat16)
            nc.scalar.dma_start(cv[:], p_cv[:])
            ones_row128 = pp.tile([1, 128], dt.bfloat16)
            nc.vector.memset(ones_row128[:], 1.0)

            with tc.tile_pool(name="ab", bufs=1) as pab:
                qT = pab.tile([128, TT], dt.bfloat16)
                kT = pab.tile([128, TT], dt.bfloat16)
                v = pab.tile([128, NT, 2, 65], dt.bfloat16)
                ctxT = pab.tile([128, TT], dt.bfloat16)

                # ---------------- stage A: LN1 stats (replicated) + QKV --------
                with (
                    tc.tile_pool(name="xtpool", bufs=1) as pxt,
                    tc.tile_pool(name="sqscr", bufs=2) as psc,
                    tc.tile_pool(name="strow", bufs=2) as pstr,
                    tc.tile_pool(name="apsum", bufs=4, space="PSUM") as pps_a,
                    tc.tile_pool(name="apsum1", bufs=1, space="PSUM") as pps_a1,
                    tc.tile_pool(name="stpsum", bufs=1, space="PSUM") as pps_st,
                ):
                    # QKV weights: [C, 128] -> [128, 8, 128] (k-tile at [:, k, :])
                    wq = pxt.tile([128, NKC, HD2], dt.bfloat16)
                    nc.scalar.dma_start(wq[:], p_wq.ap().rearrange("(k p) h -> p k h", p=128))
                    wk = pxt.tile([128, NKC, HD2], dt.bfloat16)
                    nc.scalar.dma_start(wk[:], p_wk.ap().rearrange("(k p) h -> p k h", p=128))
                    wv = pxt.tile([128, NKC, HD2], dt.bfloat16)
                    nc.scalar.dma_start(wv[:], p_wv.ap().rearrange("(k p) h -> p k h", p=128))

                    pj0 = pps_a1.tile([128, 512], dt.float32, tag="invb")
                    for _ in range(30):
                        nc.tensor.matmul(pj0[:, 0:128], ident[:], ident[:],
                                         start=True, stop=True)

                    inv_b = pxt.tile([128, TT], dt.bfloat16)
                    vT = pxt.tile([128, TT], dt.bfloat16)
                    # rows_neg [1, TT]: -mu per token (be1 == 0 so the std+eps
                    # correction row vanishes); inv_row [1, TT]: 1/(std+eps)
                    rows_neg = pxt.tile([1, TT], dt.bfloat16)
                    inv_row = pxt.tile([1, TT], dt.bfloat16)

                    # x^T resident for the QKV matmuls, DMA'd per token-chunk
                    xT = pxt.tile([128, NKC, TT], dt.bfloat16)
                    for ch in range(TT // 512):
                        nc.sync.dma_start(
                            xT[:, :, 512 * ch:512 * (ch + 1)],
                            p_xT.ap()[:, 512 * ch:512 * (ch + 1)].rearrange(
                                "(k p) t -> p k t", p=128))

                    # LN1 stats replicated on every core, computed from xT in
                    # row form: mean/meansq via ones-column PE matmul sums
                    # (no token-major x copy, no stat transposes).
                    iscb = pxt.tile([128, 1], dt.bfloat16)
                    nc.vector.memset(iscb[:], 1.0 / C)
                    nc.vector.memset(v[:, :, :, 64], 1.0)

                    for ch in range(TT // 512):
                        sl = slice(512 * ch, 512 * (ch + 1))
                        ps_mu = pps_st.tile([1, 512], dt.float32, tag="mu")
                        for k in range(NKC):
                            nc.tensor.matmul(ps_mu[:], iscb[:], xT[:, k, sl],
                                             start=(k == 0), stop=(k == NKC - 1))
                        scr2 = psc.tile([128, NKC, 512], dt.bfloat16, tag="scr")
                        nc.scalar.square(scr2[:], xT[:, :, sl])
                        ps_sq = pps_st.tile([1, 512], dt.float32, tag="sq")
                        for k in range(NKC):
                            nc.tensor.matmul(ps_sq[:], iscb[:], scr2[:, k, :],
                                             start=(k == 0), stop=(k == NKC - 1))
                        mu_row = pstr.tile([1, 512], dt.float32, tag="mur")
                        nc.vector.tensor_copy(mu_row[:], ps_mu[:])
                        nc.vector.tensor_scalar(rows_neg[:, sl], ps_mu[:], -1.0,
                                                None, alu.mult)
                        m2r = pstr.tile([1, 512], dt.float32, tag="m2r")
                        nc.vector.tensor_tensor(m2r[:], mu_row[:], mu_row[:],
                                                alu.mult)
                        varr = pstr.tile([1, 512], dt.float32, tag="varr")
                        nc.vector.tensor_tensor(varr[:], ps_sq[:], m2r[:],
                                                alu.subtract)
                        stdr = pstr.tile([1, 512], dt.float32, tag="stdr")
                        nc.scalar.activation(stdr[:], varr[:], act.Sqrt,
                                             scale=float(C) / (C - 1))
                        nc.vector.tensor_scalar(stdr[:], stdr[:], EPS, None,
                                                alu.add)
                        invr = pstr.tile([1, 512], dt.float32, tag="invr")
                        nc.vector.reciprocal_approx_fast(invr[:], stdr[:])
                        nc.vector.tensor_copy(inv_row[:, sl], invr[:])
                        # inv broadcast down partitions: 1 PE matmul + copy
                        pbi = pps_a1.tile([128, 512], dt.float32, tag="invb")
                        nc.tensor.matmul(pbi[:], ones_row128[:],
                                         inv_row[0:1, sl], start=True, stop=True)
                        nc.vector.tensor_copy(inv_b[:, sl], pbi[:])

                        # QKV for this chunk
                        for (nm, w, cw, dst) in (("q", wq, cq, qT), ("k", wk, ck, kT),
                                                 ("v", wv, cv, vT)):
                            ps = pps_a.tile([128, 512], dt.float32,
                                            name=f"ps{nm}", tag="qkv")
                            for k in range(NKC):
                                nc.tensor.matmul(ps[:], w[:, k, :], xT[:, k, sl],
                                                 start=(k == 0), stop=False)
                            nc.tensor.matmul(ps[:], cw[:], rows_neg[:, sl],
                                             start=False, stop=True)
                            nc.vector.tensor_tensor(dst[:, sl], ps[:], inv_b[:, sl],
                                                    alu.mult)

                        # v_aug [s, tile, head, 65] via PE transposes (paired)
                        for pr in range(2):
                            i0 = 4 * ch + 2 * pr
                            pvt = pps_a1.tile([128, 2, 128], dt.bfloat16, tag="vtp")
                            for u in range(2):
                                nc.tensor.transpose(
                                    pvt[:, u, :],
                                    vT[:, 128 * (i0 + u):128 * (i0 + u + 1)],
                                    ident[:])
                            nc.scalar.copy(
                                v[:, i0:i0 + 2, :, 0:64],
                                pvt[:].rearrange("p u (h d) -> p u h d", h=2))

                # ------- preload stage-C data (fills during attention) -------
                with (
                    tc.tile_pool(name="wc", bufs=1) as pwc,
                ):
                    wo_sb = pwc.tile([128, NKC, NKC, 128], dt.bfloat16)
                    for mc in range(NKC):
                        nc.scalar.dma_start(
                            wo_sb[:, mc],
                            p_woblk[mc].rearrange("(k p) c -> p k c", p=128))
                    xTs = pwc.tile([128, NKC, TS], dt.float32)
                    nc.scalar.dma_start(xTs[:], p_xTs.ap().rearrange("(k p) t -> p k t", p=128))
                    boc = pwc.tile([128, NKC], dt.float32)
                    nc.sync.dma_start(boc[:], p_boc[:])
                    b2c = pwc.tile([128, NKC], dt.float32)
                    nc.sync.dma_start(b2c[:], p_b2c[:])
                    b1c = pwc.tile([128, NMF], dt.float32)
                    nc.sync.dma_start(b1c[:], p_b1c[:])
                    isc32 = pwc.tile([128, 1], dt.float32)
                    nc.vector.memset(isc32[:], 1.0 / C)
                    iscb = pwc.tile([128, 1], dt.bfloat16)
                    nc.vector.memset(iscb[:], 1.0 / C)

                    cc_in = pdram.tile([N_CORES, 128, TS], dt.bfloat16)
                    cc_out = pdram.tile([N_CORES, 128, TS], dt.bfloat16)

                    # ---------------- stage B: attention ----------------
                    with (
                        tc.tile_pool(name="exps", bufs=6) as pexp,
                        tc.tile_pool(name="attsb", bufs=6) as pat,
                        tc.tile_pool(name="scpsum", bufs=2, space="PSUM") as pps_sc,
                        tc.tile_pool(name="ctxpsum", bufs=2, space="PSUM") as pps_ctx,
                    ):
                        for b in range(B):
                            for qt in range(T // 512):
                                G = b * T + 512 * qt
                                gsl = slice(G, G + 512)
                                nj = 4 * qt + 4
                                pc = [pps_ctx.tile([65, 512], dt.float32,
                                                   name=f"pc{h}", tag=f"ctx{h}")
                                      for h in range(2)]
                                ets = []
                                for j in range(nj):
                                    st = b * (T // 128) + j   # global s-tile index
                                    off = max(0, j - (nj - 4))
                                    o = 128 * off
                                    # both heads' scores into one 2-bank psum
                                    # tile; the two K=64 matmuls use disjoint
                                    # PE row groups and run concurrently.
                                    ps = pps_sc.tile([128, 2, 512], dt.float32,
                                                     tag="sc")
                                    for h in range(2):
                                        hsl = slice(64 * h, 64 * (h + 1))
                                        nc.tensor.matmul(
                                            ps[:, h, o:512],
                                            kT[hsl, 128 * st:128 * (st + 1)],
                                            qT[hsl, G + o:G + 512],
                                            start=True, stop=True)
                                    # one exp over both heads (amortizes the
                                    # ~300ns ACT fixed cost per instruction)
                                    et = pexp.tile([128, 2, 512], dt.bfloat16,
                                                   tag="et")
                                    nc.scalar.activation(
                                        et[:, :, o:512], ps[:, :, o:512],
                                        act.Exp, scale=1.0 / float(np.sqrt(H)))
                                    if j >= nj - 4:
                                        for h in range(2):
                                            nc.vector.tensor_tensor(
                                                et[:, h, o:o + 128],
                                                et[:, h, o:o + 128],
                                                maskd[:], alu.mult)
                                    ets.append((et, o))
                                    # software pipeline: AV for tile j-1 after scores of j
                                    if j > 0:
                                        pe2, po = ets[j - 1]
                                        for h in range(2):
                                            nc.tensor.matmul(
                                                pc[h][:, po:512],
                                                v[:, b * (T // 128) + j - 1, h, :],
                                                pe2[:, h, po:512],
                                                start=(j - 1 == 0), stop=False)
                                pe2, po = ets[nj - 1]
                                for h in range(2):
                                    nc.tensor.matmul(
                                        pc[h][:, po:512],
                                        v[:, b * (T // 128) + nj - 1, h, :],
                                        pe2[:, h, po:512],
                                        start=(nj == 1), stop=True)
                                # normalize by Z (row 64 of each ctx psum).
                                # zbf row = [1/Z_h0 | 1/Z_h1]; broadcast must
                                # write from partition 0, so head h reads
                                # zbf[64h:64h+64, 512h:512h+512].
                                zr = pat.tile([1, 2, 512], dt.float32, tag="zr")
                                nc.vector.tensor_copy(zr[:, 0, :], pc[0][64:65, :])
                                nc.vector.tensor_copy(zr[:, 1, :], pc[1][64:65, :])
                                zi = pat.tile([1, 2, 512], dt.float32, tag="zi")
                                nc.vector.reciprocal_approx_fast(zi[:], zr[:])
                                zib = pat.tile([1, 1024], dt.bfloat16, tag="zib")
                                nc.vector.tensor_copy(
                                    zib[:].rearrange("p (h t) -> p h t", h=2), zi[:])
                                zbf = pat.tile([128, 1024], dt.bfloat16, tag="zb")
                                nc.gpsimd.partition_broadcast(zbf[:], zib[0:1, :])
                                for h in range(2):
                                    nc.vector.tensor_tensor(
                                        ctxT[64 * h:64 * (h + 1), gsl],
                                        pc[h][0:64, :],
                                        zbf[64 * h:64 * (h + 1),
                                            512 * h:512 * (h + 1)],
                                        alu.mult)
                                # ship this token-chunk to its owner core
                                nc.sync.dma_start(cc_in[b * 4 + qt], ctxT[:, gsl])

                    # ---------------- AllToAll: heads -> tokens ----------------
                    nc.gpsimd.collective_compute(
                        "AllToAll", alu.bypass,
                        replica_groups=[list(range(N_CORES))],
                        ins=[cc_in.opt()],
                        outs=[cc_out.opt()],
                    )

                    # ---------------- stage C: Wo + LN2 + FFN ----------------
                    with (
                        tc.tile_pool(name="postsb", bufs=1) as pq,
                        tc.tile_pool(name="wstream", bufs=3) as pw,
                        tc.tile_pool(name="evict", bufs=3) as pev,
                        tc.tile_pool(name="ln2tmp", bufs=1) as pl2,
                        tc.tile_pool(name="ffpsum", bufs=4, space="PSUM") as pps_ff,
                        tc.tile_pool(name="cpsum", bufs=1, space="PSUM") as pps_c,
                    ):
                        pj = pps_ff.tile([128, 128], dt.float32, tag="ff")
                        for _ in range(128):
                            nc.tensor.matmul(pj[:], ident[:], ident[:],
                                             start=True, stop=True)
                        # preload the sqrt act-table during the A2A wait so the
                        # LN2 sqrt doesn't pay the ~2.7us reload mid-stage-C
                        sq0 = pq.tile([1, 16], dt.float32)
                        nc.vector.memset(sq0[:], 1.0)
                        sq1 = pq.tile([1, 16], dt.float32)
                        nc.scalar.activation(sq1[:], sq0[:], act.Sqrt)

                        ctxF = pq.tile([128, NKC, TS], dt.bfloat16)
                        for j in range(N_CORES):
                            eng = nc.sync if j % 2 == 0 else nc.scalar
                            eng.dma_start(ctxF[:, j, :], cc_out[j])

                        r2T = pq.tile([128, NKC, TS], dt.float32)
                        sq = pl2.tile([128, NKC, TS], dt.bfloat16)
                        ps1 = pps_c.tile([1, TS], dt.float32, tag="s1")
                        ps2 = pps_c.tile([1, TS], dt.float32, tag="s2")
                        for mc in range(NKC):
                            ps = pps_ff.tile([128, TS], dt.float32, tag="ff")
                            for k in range(NKC):
                                nc.tensor.matmul(ps[:], wo_sb[:, mc, k, :],
                                                 ctxF[:, k, :],
                                                 start=(k == 0), stop=(k == NKC - 1))
                            nc.vector.scalar_tensor_tensor(
                                r2T[:, mc, :], ps[:], boc[:, mc:mc + 1],
                                xTs[:, mc, :], alu.add, alu.add)
                            nc.scalar.square(sq[:, mc, :], r2T[:, mc, :])
                            nc.tensor.matmul(ps1[:], isc32[:], r2T[:, mc, :],
                                             start=(mc == 0), stop=(mc == NKC - 1))
                            nc.tensor.matmul(ps2[:], iscb[:], sq[:, mc, :],
                                             start=(mc == 0), stop=(mc == NKC - 1))

                        # ---- LN2 over the channel (partition) dim ----
                        mu2f = pl2.tile([1, TS], dt.float32)
                        nc.vector.tensor_copy(mu2f[:], ps1[:])
                        varf2 = pl2.tile([1, TS], dt.float32)
                        nc.vector.tensor_tensor(varf2[:], mu2f[:], mu2f[:], alu.mult)
                        nc.vector.tensor_tensor(varf2[:], ps2[:], varf2[:],
                                                alu.subtract)
                        stdf2 = pl2.tile([1, TS], dt.float32)
                        nc.scalar.activation(stdf2[:], varf2[:], act.Sqrt,
                                             scale=float(C) / (C - 1))
                        nc.vector.tensor_scalar(stdf2[:], stdf2[:], EPS, None,
                                                alu.add)
                        inv2 = pl2.tile([1, TS], dt.float32)
                        nc.vector.reciprocal_approx_fast(inv2[:], stdf2[:])
                        mu2row = pl2.tile([1, TS], dt.bfloat16)
                        nc.vector.tensor_copy(mu2row[:], mu2f[:])
                        inv2row = pl2.tile([1, TS], dt.bfloat16)
                        nc.vector.tensor_copy(inv2row[:], inv2[:])
                        pmb = pps_c.tile([128, TS], dt.float32, tag="bc1")
                        nc.tensor.matmul(pmb[:], ones_row128[:], mu2row[0:1, :],
                                         start=True, stop=True)
                        m2b = pl2.tile([128, TS], dt.bfloat16)
                        nc.vector.tensor_copy(m2b[:], pmb[:])
                        pib = pps_c.tile([128, TS], dt.float32, tag="bc2")
                        nc.tensor.matmul(pib[:], ones_row128[:], inv2row[0:1, :],
                                         start=True, stop=True)
                        i2b = pl2.tile([128, TS], dt.bfloat16)
                        nc.vector.tensor_copy(i2b[:], pib[:])

                        xn2T = pq.tile([128, NKC, TS], dt.bfloat16)
                        for mc in range(NKC):
                            tmp = pev.tile([128, TS], dt.bfloat16, tag="xtmp")
                            eng = nc.gpsimd if mc % 2 else nc.vector
                            eng.tensor_tensor(tmp[:], r2T[:, mc, :], m2b[:],
                                              alu.subtract)
                            nc.vector.tensor_tensor(xn2T[:, mc, :], tmp[:], i2b[:],
                                                    alu.mult)

                        # ---- FFN ----
                        hT = pq.tile([128, NMF, TS], dt.bfloat16)
                        for mf in range(NMF):
                            w1_blk = pw.tile([128, NKC, 128], dt.bfloat16, tag="w1")
                            nc.sync.dma_start(
                                w1_blk[:],
                                p_w1blk[mf].rearrange("(k p) f -> p k f", p=128))
                            ps = pps_ff.tile([128, TS], dt.float32, tag="ff")
                            for k in range(NKC):
                                nc.tensor.matmul(ps[:], w1_blk[:, k, :],
                                                 xn2T[:, k, :],
                                                 start=(k == 0), stop=(k == NKC - 1))
                            nc.scalar.activation(hT[:, mf, :], ps[:],
                                                 act.Relu,
                                                 bias=b1c[:, mf:mf + 1])

                        for mc in range(NKC):
                            w2_blk = pw.tile([128, NMF, 128], dt.bfloat16, tag="w2")
                            nc.sync.dma_start(
                                w2_blk[:],
                                p_w2blk[mc].rearrange("(k p) c -> p k c", p=128))
                            ps = pps_ff.tile([128, TS], dt.float32, tag="ff")
                            for k in range(NMF):
                                nc.tensor.matmul(ps[:], w2_blk[:, k, :], hT[:, k, :],
                                                 start=(k == 0), stop=(k == NMF - 1))
                            ot = pev.tile([128, TS], dt.float32, tag="ot")
                            nc.vector.scalar_tensor_tensor(
                                ot[:], ps[:], b2c[:, mc:mc + 1], r2T[:, mc, :],
                                alu.add, alu.add)
                            nc.sync.dma_start(p_out[128 * mc:128 * (mc + 1), :], ot[:])

    nc.compile()
    return nc


def _host_prep(inputs):
    """Fold layernorm affine params into weights; build per-core input maps."""
    x = np.asarray(inputs["x"], np.float32)
    Wq = np.asarray(inputs["Wq"], np.float32)
    Wk = np.asarray(inputs["Wk"], np.float32)
    Wv = np.asarray(inputs["Wv"], np.float32)
    Wo = np.asarray(inputs["Wo"], np.float32)
    bo = np.asarray(inputs["bo"], np.float32)
    W1 = np.asarray(inputs["W1"], np.float32)
    b1 = np.asarray(inputs["b1"], np.float32)
    W2 = np.asarray(inputs["W2"], np.float32)
    b2 = np.asarray(inputs["b2"], np.float32)
    g1 = np.asarray(inputs["g1"], np.float32)
    be1 = np.asarray(inputs["be1"], np.float32)
    g2 = np.asarray(inputs["g2"], np.float32)
    be2 = np.asarray(inputs["be2"], np.float32)

    xf = x.reshape(TT, C)                      # both batches stacked
    xT = np.ascontiguousarray(xf.T)            # [C, TT]

    # the kernel folds LN1 as (x@Weff - mu*colsum(Weff)) * inv; the be1 bias
    # term would need a second correction row — this problem has be1 == 0.
    assert np.abs(be1).max() == 0.0

    def fold_qkv(W):
        Weff = g1[:, None] * W                  # [NH, C, H] with g1 on C
        Weff = np.ascontiguousarray(np.transpose(Weff, (1, 0, 2)))  # [C, NH, H]
        colsum = Weff.sum(axis=0)               # [NH, H]
        return Weff, colsum

    Wq_e, csq = fold_qkv(Wq)
    Wk_e, csk = fold_qkv(Wk)
    Wv_e, csv = fold_qkv(Wv)

    woT = np.ascontiguousarray(Wo.T)            # [NH*H, C]
    w1T = np.ascontiguousarray(g2[:, None] * W1.T)   # [C, FF]
    b1_eff = b1 + be2 @ W1.T                         # [FF]
    w2T = np.ascontiguousarray(W2.T)            # [FF, C]

    # blocked weights: [nblocks, K, 128] with contiguous [K, 128] blocks
    woblk = np.ascontiguousarray(
        woT.reshape(C, NKC, 128).transpose(1, 0, 2))
    w1blk = np.ascontiguousarray(
        w1T.reshape(C, NMF, 128).transpose(1, 0, 2))
    w2blk = np.ascontiguousarray(
        w2T.reshape(FF, NKC, 128).transpose(1, 0, 2))

    tq = np.arange(128)[None, :]
    s = np.arange(128)[:, None]
    maskd = (s <= tq).astype(BF16)

    shared = {
        "p_xT": xT.astype(BF16),
        "p_woblk": woblk.astype(BF16),
        "p_boc": np.ascontiguousarray(
            bo.reshape(NKC, 128).T).astype(np.float32),
        "p_w1blk": w1blk.astype(BF16),
        "p_b1c": np.ascontiguousarray(
            b1_eff.reshape(NMF, 128).T).astype(np.float32),
        "p_w2blk": w2blk.astype(BF16),
        "p_b2c": np.ascontiguousarray(
            b2.reshape(NKC, 128).T).astype(np.float32),
        "p_maskd": maskd,
        "p_ident": np.eye(128, dtype=np.float32).astype(BF16),
    }

    in_maps = []
    for r in range(N_CORES):
        h0 = HPC * r
        hs = slice(h0, h0 + HPC)
        b_r, s_r = divmod(r, N_CORES // B)
        tok = slice(s_r * TS, (s_r + 1) * TS)
        xTs = np.ascontiguousarray(x[b_r].T[:, tok])
        m = dict(shared)
        m["p_xTs"] = xTs.astype(np.float32)
        m["p_wq"] = np.ascontiguousarray(
            Wq_e[:, hs, :].reshape(C, HD2)).astype(BF16)
        m["p_wk"] = np.ascontiguousarray(
            Wk_e[:, hs, :].reshape(C, HD2)).astype(BF16)
        m["p_wv"] = np.ascontiguousarray(
            Wv_e[:, hs, :].reshape(C, HD2)).astype(BF16)
        m["p_cq"] = csq[hs].reshape(1, HD2).astype(BF16)
        m["p_ck"] = csk[hs].reshape(1, HD2).astype(BF16)
        m["p_cv"] = csv[hs].reshape(1, HD2).astype(BF16)
        in_maps.append(m)
    return in_maps


def kernel(**inputs) -> np.ndarray:
    from concourse.bass_utils import run_bass_kernel_spmd

    if "nc" not in _BUILT:
        _BUILT["nc"] = _build()
    nc = _BUILT["nc"]

    in_maps = _host_prep(inputs)
    res = run_bass_kernel_spmd(nc, in_maps, core_ids=list(range(N_CORES)))

    out = np.empty((B, T, C), np.float32)
    for r in range(N_CORES):
        b_r, s_r = divmod(r, N_CORES // B)
        out[b_r, s_r * TS:(s_r + 1) * TS, :] = res.results[r]["p_out"].T
    return out
